# revision 48
# baseline (speedup 1.0000x reference)
"""Distributed attention kernel for Trainium2 (8 NeuronCores).

Problem: B=2, N=2048, DIM=1024, 16 heads x 64 dims.
  qkv = hidden @ w_qkv ; per-head RMSNorm(q,k) ; RoPE ; softmax attention
  (bf16 matmuls, fp32 accumulation) ; out = attn @ w_out.

Sharding: tensor-parallel over heads. Core c owns heads {2c, 2c+1}.
Each core computes its 2 heads' attention plus its partial out-projection
(128 rows of w_out); host sums the 8 partial outputs (f32).

Per-core layout strategy (feature-major q/k, "S^T" scores):
  - Host passes hidden^T [1024, 4096] (bf16), so QKV matmuls produce
    q^T,k^T,v^T feature-major [128, 4096] directly (w slice as stationary).
  - RMSNorm over head dims (partition axis) via block-ones matmul (sum of
    squares broadcast to all 128 partitions) + exp(-0.5*ln(x)) on ScalarE
    (ACT Rsqrt/Reciprocal are banned; Ln+Exp share one table set).
  - RoPE: q*cos + rot_half(q)*sin, with rot_half folded into a constant
    128x128 permutation/sign matmul; sin table pre-permuted on host.
  - Scores computed transposed: S^T[k,q] tiles via lhsT=k^T chunk,
    rhs=q^T chunk; the two heads packed via PE row-tiling (K=64 each).
  - softmax without max subtraction (normed+roped logits are O(1)); exp on
    ScalarE reading 2 PSUM banks per instruction.
  - PV: stationary = [ones(64) | v(64)] so PSUM rows 0:64 get the softmax
    denominator (replicated) and rows 64:128 get unnormalized attn^T.
    1/denom = exp(-ln(denom)) on ACT; attnT = attn * (1/denom) on DVE.
  - out-proj: lhsT = attn^T (both heads = K=128), rhs = w_out rows slice.

Scheduling: flat software pipeline over global iterations g (one
exp-chunk per g; step s = (batch, q-chunk), g0 = 16*s):
  produce: score matmuls + exp at g0+kc      (ACT-paced)
  pv:      PV accumulation at g0+kc+2, staged to SBUF at g0+18,
           normalized at g0+19
  po:      out-projection + DMA at g0+21..29
Per-iteration emission order is pv, po, unit, produce: the PE executes in
order, so ready consume-work must sit ahead of score matmuls that may
stall on a free sAB PSUM slot — PE density keeps the p-state at full
clock (2.4 GHz vs 1.2 when gappy).  QKV "units" are pinned at explicit g
so each lands before its first consumer (in-order PE would deadlock
otherwise); batch 1's units hide under batch 0's attention windows.
Inputs ride three DMA queues in parallel (sync / ACT / gpsimd-SWDGE).
"""

import os
import numpy as np
import ml_dtypes
from contextlib import ExitStack

import concourse.bass as bass
import concourse.mybir as mybir
import concourse.tile as tile
from concourse.vector_clock import VectorClock, ScopedClock
from concourse.bass_utils import run_bass_kernel_spmd


def _patched_drain_and_barrier(self, tick_clock, wait_clock):
    """Tile's exit drain puts one wait per live semaphore on a single
    instruction; this walrus build rejects >N waits per instruction
    ("Too many sync wait commands"). Split into one drain per proc."""
    nc = self.nc
    gc = tick_clock.global_clock
    n = len(gc)
    for p in range(n):
        if gc[p] > 0:
            vc = VectorClock([gc[p] if i == p else 0 for i in range(n)])
            d = nc.sync.drain()
            wait_clock.add_sem_waits(d.ins, ScopedClock({None: vc}))
    nc.all_engine_barrier()
    popped = nc._tile_sem_poison_stack.pop()
    assert popped is self._sem_poison
    nc.clear_and_free_semaphores(list(self.sems.allocated().values()))
    nc.all_engine_barrier()


tile.TileContext._drain_and_barrier = _patched_drain_and_barrier

_TPB_ENGINES = {
    mybir.EngineType.PE, mybir.EngineType.Activation, mybir.EngineType.Pool,
    mybir.EngineType.DVE, mybir.EngineType.SP,
}


def _fixup_wait_limits(nc):
    """Walrus here encodes at most 2 sync waits per instruction (1 for
    activations, which also get a table-load wait). Offload excess waits
    onto same-engine NoOps inserted immediately before the instruction."""
    import bass_rust

    f = nc.m.functions[0]
    for blk in f.blocks:
        insts = blk.instructions
        i = 0
        while i < len(insts):
            inst = insts[i]
            si = inst.sync_info
            waits = list(si.on_wait) if (si and si.on_wait) else []
            limit = 1
            if len(waits) > limit:
                if inst.engine not in _TPB_ENGINES:
                    i += 1
                    continue
                excess, keep = waits[:-limit], waits[-limit:]
                ups = list(si.on_update) if si.on_update else []
                inst.sync_info = mybir.SyncInfo(on_wait=keep, on_update=ups)
                pos = i
                for j in range(0, len(excess)):
                    ni = bass_rust.InstNoOp(
                        name=nc.get_next_instruction_name(), ins=[], outs=[])
                    ni.engine = inst.engine
                    ni.sync_info = mybir.SyncInfo(
                        on_wait=excess[j:j + 1], on_update=[])
                    nc.register_instruction(ni)
                    insts.insert(pos, ni)
                    pos += 1
                    i += 1
            i += 1

BF16 = mybir.dt.bfloat16
F32 = mybir.dt.float32
AF = mybir.ActivationFunctionType
ALU = mybir.AluOpType

DIM = 1024
NHEADS = 16
HD = 64
B = 2
N = 2048
T = B * N            # 4096 tokens
NCORES = 8
TC = 512             # token chunk (free dim of most tiles)
NTC = T // TC        # 8 token chunks
NQC = N // TC        # 4 q chunks per batch
KCH = 128            # k-position chunk (PV contraction)
NKC = N // KCH       # 16 k chunks per batch

_CACHE = {}


def build_graph():
    nc = bass.Bass()
    # register 1e-6 as a const AP so activation(bias=1e-6) lowers the same
    # way as the built-in 0.0/1.0 consts (tile-AP biases break walrus here)
    _epst = nc.alloc_sbuf_tensor("const-float32-eps", [128, 1], F32)
    nc.gpsimd.memset(_epst.ap(), 1e-6)
    nc.const_aps.aps[(F32, 1e-6)] = _epst.ap()
    nc.all_engine_barrier()
    # ht packed on host: [tcix] -> [128, 8*512] (fc-major columns)
    ht = nc.declare_dram_parameter("ht", [NTC, 128, 8 * TC], BF16,
                                   isOutput=False)
    wq = nc.declare_dram_parameter("wq", [128, 1024], BF16, isOutput=False)
    wk = nc.declare_dram_parameter("wk", [128, 1024], BF16, isOutput=False)
    wv = nc.declare_dram_parameter("wv", [128, 1024], BF16, isOutput=False)
    wo = nc.declare_dram_parameter("wo", [128, DIM], BF16, isOutput=False)
    cosw = nc.declare_dram_parameter("cosw", [128, N], BF16, isOutput=False)
    sinpw = nc.declare_dram_parameter("sinpw", [128, N], BF16, isOutput=False)
    rotm = nc.declare_dram_parameter("rotm", [128, 128], BF16, isOutput=False)
    ident = nc.declare_dram_parameter("ident", [128, 128], BF16, isOutput=False)
    ssqq = nc.declare_dram_parameter("ssqq", [128, 128], BF16, isOutput=False)
    ssqk = nc.declare_dram_parameter("ssqk", [128, 128], BF16, isOutput=False)
    outp = nc.declare_dram_parameter("out", [T, DIM], BF16, isOutput=True)

    with ExitStack() as ctx:
        tc_ = ctx.enter_context(tile.TileContext(nc))
        singles = ctx.enter_context(tc_.tile_pool(name="singles", bufs=1))
        work = ctx.enter_context(tc_.tile_pool(name="work", bufs=2))
        big = ctx.enter_context(tc_.tile_pool(name="big", bufs=1))

        # ---- load constants/weights ----
        # Each DMA rides a single ~40-60 GB/s queue; the 8 SWDGE (gpsimd)
        # queues round-robin by emission order.  Split every 1MB ht tile
        # into two half-tile DMAs on different queues, and give cos/sin
        # their own SWDGE queues, so the prologue-critical loads all land
        # by ~17us.  Sync queue carries weights + tiny tables in use-order.
        cos_s = singles.tile([128, N], BF16, tag="cos", name="cos_s")
        sinp_s = singles.tile([128, N], BF16, tag="sinp", name="sinp_s")

        def load_ht(bix, extras=()):
            tiles = {}
            for tcl in range(4):
                tcix = 4 * bix + tcl
                t = big.tile([128, 8 * TC], BF16, tag=f"ht_{tcl}",
                             bufs=1, name=f"htt{tcix}")
                h = 4 * TC
                nc.gpsimd.dma_start(out=t[:, 0:h], in_=ht[tcix, :, 0:h])
                nc.gpsimd.dma_start(out=t[:, h:2 * h], in_=ht[tcix, :, h:2 * h])
                for x in extras:
                    x()
                    extras = ()
                for fc in range(8):
                    tiles[(fc, tcix)] = t[:, fc * TC:(fc + 1) * TC]
            return tiles

        ht_b0 = load_ht(0, extras=(
            lambda: nc.gpsimd.dma_start(out=cos_s[:], in_=cosw[:, :]),
            lambda: nc.gpsimd.dma_start(out=sinp_s[:], in_=sinpw[:, :]),
        ))
        w_s = {}
        for name, prm in (("wk", wk), ("wq", wq)):
            t = singles.tile([128, 1024], BF16, tag=f"{name}all",
                             name=f"{name}s")
            nc.sync.dma_start(out=t[:], in_=prm[:, :])
            w_s[name] = [t[:, fc * 128:(fc + 1) * 128] for fc in range(8)]
        rot_s = singles.tile([128, 128], BF16, tag="rot", name="rot_s")
        nc.sync.dma_start(out=rot_s[:], in_=rotm[:, :])
        id_s = singles.tile([128, 128], BF16, tag="id", name="id_s")
        nc.sync.dma_start(out=id_s[:], in_=ident[:, :])
        ssq_s = {}
        for name, prm in (("k", ssqk), ("q", ssqq)):
            t = singles.tile([128, 128], BF16, tag=f"ssq{name}",
                             name=f"ssq_s{name}")
            nc.sync.dma_start(out=t[:], in_=prm[:, :])
            ssq_s[name] = t
        t = singles.tile([128, 1024], BF16, tag="wvall", name="wvs")
        nc.sync.dma_start(out=t[:], in_=wv[:, :])
        w_s["wv"] = [t[:, fc * 128:(fc + 1) * 128] for fc in range(8)]
        wo_s = singles.tile([128, DIM], BF16, tag="wo", name="wo_s")
        nc.sync.dma_start(out=wo_s[:], in_=wo[:, :])

        # persistent activations
        qT = singles.tile([128, T], BF16, tag="qT", name="qT")
        kT = singles.tile([128, T], BF16, tag="kT", name="kT")
        vT = singles.tile([128, T], BF16, tag="vT", name="vT")
        # v token-major, per (batch, head): [ones(64) | v(64)] per 128-chunk
        v_sb = {}
        for b in range(B):
            for h in range(2):
                t = singles.tile([128, NKC * 128], BF16, tag=f"vsb{b}{h}",
                                 name=f"vsb{b}{h}")
                nc.gpsimd.memset(t[:], 1.0)
                v_sb[(b, h)] = t
        # preload the Ln/Exp table set off the critical path
        warm = singles.tile([128, 1], F32, tag="warm", name="warm")
        nc.scalar.activation(warm[:], nc.const_aps.tensor(1.0, (128, 1), F32),
                             AF.Ln)

        # Single psum pool.
        # tagA [128,1024] f32 bufs=2: qkv unit psums / sAB score tiles (4 bk)
        # tagB [128,1024] f32 bufs=1: pv accumulator                   (2 bk)
        # tagC [128, 512] f32 bufs=2: ssqb, qf, vtrans, out-proj po    (2 bk)
        pp = ctx.enter_context(
            tc_.tile_pool(name="pp", bufs=1, space="PSUM"))
        inv_sqrt_hd = float(1.0 / np.sqrt(HD))

        ht_tiles = {0: ht_b0}

        def unit(which, tcix, veng=None):
            """One QKV projection unit: q, k or v for one 512-token chunk.
            veng picks the engine for the elementwise mults: Pool (default)
            keeps them off DVE, but Pool multiplies run at 0.42 efficiency,
            so unit-dense phases pass nc.vector instead."""
            if veng is None:
                veng = nc.gpsimd
            bix = tcix // 4
            ht_b = ht_tiles[bix]
            tsl = slice(tcix * TC, (tcix + 1) * TC)
            psl = _postbl(tsl)
            ps = pp.tile([128, 2 * TC], F32, tag="tagA", bufs=2,
                         name=f"ps_{which}{tcix}")
            for fc in range(8):
                nc.tensor.matmul(
                    ps[:, 0:TC], w_s["w" + which][fc],
                    ht_b[(fc, tcix)], start=(fc == 0), stop=(fc == 7))
            if which == "v":
                nc.vector.tensor_copy(vT[:, tsl], ps[:, 0:TC])
                # transpose this chunk's v to token-major now
                for kc in range(4 * (tcix % 4), 4 * (tcix % 4) + 4):
                    t2 = slice(bix * N + kc * KCH,
                               bix * N + (kc + 1) * KCH)
                    pt = pp.tile([128, 128], BF16, tag="tagC",
                                 bufs=2, name="vtp")
                    nc.tensor.transpose(pt[:], vT[:, t2], id_s[:])
                    nc.vector.tensor_copy(
                        v_sb[(bix, 0)][:, kc * 128 + 64:
                                       kc * 128 + 128], pt[:, 0:64])
                    nc.vector.tensor_copy(
                        v_sb[(bix, 1)][:, kc * 128 + 64:
                                       kc * 128 + 128], pt[:, 64:128])
                return
            raw = work.tile([128, TC], BF16, tag="raw", name="raw")
            nc.vector.tensor_copy(raw[:], ps[:, 0:TC])
            sq = work.tile([128, TC], BF16, tag="sq", name="sq")
            veng.tensor_tensor(sq[:], raw[:], raw[:], ALU.mult)
            ssqb = pp.tile([128, TC], F32, tag="tagC", bufs=2,
                           name="ssqb")
            nc.tensor.matmul(ssqb[:], ssq_s[which][:], sq[:],
                             start=True, stop=True)
            # scale = rsqrt(ssq/HD + eps) = exp(-0.5*ln(...))
            lnb = work.tile([128, TC], F32, tag="lnb", name="lnb")
            nc.scalar.activation(lnb[:], ssqb[:], AF.Ln,
                                 bias=1e-6, scale=1.0 / HD)
            scale = work.tile([128, TC], BF16, tag="scale",
                              name="scale")
            nc.scalar.activation(scale[:], lnb[:], AF.Exp, scale=-0.5)
            # rope on unscaled q; per-token scale in the final mult
            qs_cos = work.tile([128, TC], BF16, tag="qs_cos",
                               name="qs_cos")
            veng.tensor_tensor(qs_cos[:], raw[:], cos_s[:, psl],
                               ALU.mult)
            qs_sin = work.tile([128, TC], BF16, tag="qs_sin",
                               name="qs_sin")
            veng.tensor_tensor(qs_sin[:], raw[:], sinp_s[:, psl],
                               ALU.mult)
            qf = pp.tile([128, TC], F32, tag="tagC", bufs=2, name="qf")
            nc.tensor.matmul(qf[:], id_s[:], qs_cos[:],
                             start=True, stop=False)
            nc.tensor.matmul(qf[:], rot_s[:], qs_sin[:],
                             start=False, stop=True)
            dst = qT if which == "q" else kT
            nc.vector.tensor_tensor(dst[:, tsl], qf[:], scale[:], ALU.mult)

        # ---------------- pipeline stages ----------------
        eABs = {}     # (b, qc) -> eAB tile
        attnTs = {}   # (b, qc) -> attnT tile

        def make_produce(b, qc):
            """Generator: score matmuls + exp for (b, qc), one kc per next()."""
            qsl = slice(b * N + qc * TC, b * N + (qc + 1) * TC)
            eAB = big.tile([128, 2 * NKC * TC], BF16, tag="eAB", bufs=2,
                           name=f"eAB{b}{qc}")
            eABs[(b, qc)] = eAB
            for kc in range(NKC):
                sAB = pp.tile([128, 2 * TC], F32, tag="tagA",
                              bufs=2, name="sAB")
                ksl = slice(b * N + kc * KCH, b * N + (kc + 1) * KCH)
                nc.tensor.matmul(
                    sAB[:, 0:TC],
                    kT[0:64, ksl], qT[0:64, qsl],
                    start=True, stop=True, tile_position=(0, 0))
                nc.tensor.matmul(
                    sAB[:, TC:2 * TC],
                    kT[64:128, ksl], qT[64:128, qsl],
                    start=True, stop=True, tile_position=(64, 0))
                esl = slice(kc * 2 * TC, (kc + 1) * 2 * TC)
                nc.scalar.activation(eAB[:, esl], sAB[:], AF.Exp,
                                     scale=inv_sqrt_hd)
                yield

        def make_pv(b, qc):
            """Generator: PV accumulation (2 matmuls per next()); the PSUM
            result is staged to SBUF with the last pv pair (frees the single
            tagB accumulator early), and a later next() normalizes.  The
            final step reads PSUM directly: no successor needs the
            accumulator, and skipping the staging shortens the tail chain."""
            tail = (b, qc) == (B - 1, NQC - 1)
            eAB = eABs[(b, qc)]
            pv = pp.tile([128, 2 * TC], F32, tag="tagB", bufs=1,
                         name="pv")
            dnm = atn = None
            for kc in range(NKC):
                for h in (0, 1):
                    nc.tensor.matmul(
                        pv[:, h * TC:(h + 1) * TC],
                        v_sb[(b, h)][:, kc * 128:(kc + 1) * 128],
                        eAB[:, (2 * kc + h) * TC:(2 * kc + h + 1) * TC],
                        start=(kc == 0), stop=(kc == NKC - 1),
                        skip_group_check=True)
                if kc == NKC - 1 and not tail:
                    # stage both PSUM halves to SBUF at base partition 0
                    # immediately (SB+SB tensor ops require equal base
                    # partitions; staging also frees tagB early)
                    dnm = work.tile([64, 2 * TC], F32, tag="dnm", bufs=1,
                                    name="dnm")
                    nc.vector.tensor_copy(dnm[:], pv[0:64, :])
                    atn = work.tile([64, 2 * TC], F32, tag="atn", bufs=1,
                                    name="atn")
                    nc.vector.tensor_copy(atn[:], pv[64:128, :])
                yield
            yield
            # normalization: 1/denom = exp(-ln(denom)) on ACT (no divide op
            # exists on DVE/Pool; ACT Reciprocal is banned), then DVE mults.
            lnd = work.tile([64, 2 * TC], F32, tag="lnd", bufs=1, name="lnd")
            nc.scalar.activation(lnd[:], pv[0:64, :] if tail else dnm[:],
                                 AF.Ln)
            rbc = work.tile([64, 2 * TC], F32, tag="rbc", bufs=1, name="rbc")
            nc.scalar.activation(rbc[:], lnd[:], AF.Exp, scale=-1.0)
            attnT = work.tile([128, TC], BF16, tag="attnT", bufs=3,
                              name="attnT")
            attnTs[(b, qc)] = attnT
            if tail:
                a0, a1 = pv[64:128, 0:TC], pv[64:128, TC:2 * TC]
            else:
                a0, a1 = atn[0:64, 0:TC], atn[0:64, TC:2 * TC]
            nc.vector.tensor_tensor(
                attnT[0:64, :], a0, rbc[0:64, 0:TC], ALU.mult)
            nc.vector.tensor_tensor(
                attnT[64:128, :], a1, rbc[0:64, TC:2 * TC], ALU.mult)
            yield

        def make_po(b, qc):
            """Generator: out-projection, one (mt, nn) matmul per next();
            ob staging copies on DVE; DMA per mt."""
            tail = (b, qc) == (B - 1, NQC - 1)
            attnT = attnTs.pop((b, qc))
            for mt in range(TC // 128):
                ob = work.tile([128, DIM], BF16, tag="ob", bufs=3,
                               name="ob")
                for nn in range(DIM // TC):
                    # in the tail, tagA's 4 banks are dead: alternate slots
                    # so the po matmuls outrun their staging copies
                    tg = "tagA" if (tail and nn == 1) else "tagC"
                    po = pp.tile([128, TC], F32, tag=tg, bufs=2,
                                 name="po")
                    nc.tensor.matmul(
                        po[:], attnT[:, mt * 128:(mt + 1) * 128],
                        wo_s[:, nn * TC:(nn + 1) * TC],
                        start=True, stop=True)
                    nc.vector.tensor_copy(
                        ob[:, nn * TC:(nn + 1) * TC], po[:])
                    yield
                r0 = b * N + qc * TC + mt * 128
                # final step: the ACT hwdge queue is idle once the exps are
                # done — split the closing 1MB of output across both queues
                # so the end-of-kernel barrier waits half as long
                if tail and mt % 2 == 1:
                    nc.scalar.dma_start(out=outp[r0:r0 + 128, :], in_=ob[:])
                else:
                    nc.sync.dma_start(out=outp[r0:r0 + 128, :], in_=ob[:])
            eABs.pop((b, qc), None)

        def drain(gen):
            if gen is not None:
                for _ in gen:
                    pass

        # ---------------- schedule ----------------
        # prologue: only k0 + q0 (produce(0,0) kc 0..3 need just the first
        # 512 k-positions); k1-k3 are interleaved ahead of the score
        # chunks that need them so exp work starts as early as possible.
        for u in (("k", 0), ("q", 0)):
            unit(*u, veng=nc.vector if g < 80 else None)
        # batch 1 hidden loads: emitted now so the DMAs sit ahead of the
        # output DMAs in the queue; they fire once b0's units release slots
        ht_tiles[1] = load_ht(1)

        # Flat schedule over global iterations g (one exp-chunk per g).
        # Per (b, qc) step s (g0 = 16*s): produce kc at g0+kc; pv kc at
        # g0+kc+2 (2-iteration slack, matching the 2 sAB buffers); the
        # pv->SBUF stage at g0+18, normalization at g0+19; out-projection
        # pieces at g0+21..28.  QKV units are pinned at explicit g so every
        # PE instruction only ever waits on earlier-emitted work, and each
        # unit lands before its first consumer (in-order PE = deadlock
        # otherwise).  Emission order within one g: pv, po, unit, produce —
        # ready consume-work must sit ahead of score matmuls that may stall
        # on a free sAB slot.
        BQ = [(b, qc) for b in range(B) for qc in range(NQC)]
        events = {}

        def at(g, kind, item):
            events.setdefault(g, {}).setdefault(kind, []).append(item)

        for s, (b, qc) in enumerate(BQ):
            g0 = 16 * s
            pg = make_produce(b, qc)
            vg = make_pv(b, qc)
            og = make_po(b, qc)
            for kc in range(NKC):
                at(g0 + kc, "produce", pg)
                # step 0's pv chain races the just-emitted v units: one
                # extra iteration of slack there
                at(g0 + kc + (3 if s == 0 else 2), "pv", vg)
            at(g0 + 18, "pv", vg)
            at(g0 + 19, "pv", vg)
            # 9 pulls: 8 matmul+copy pieces plus one more to resume the
            # generator past its last yield (emits the final mt3 DMA)
            for j in range(9):
                at(g0 + 21 + j, "po", og)
        USCHED = [
            (1, ("v", 0)), (2, ("k", 1)), (4, ("v", 1)), (6, ("k", 2)),
            (8, ("v", 2)), (10, ("k", 3)), (12, ("v", 3)), (14, ("q", 1)),
            (16, ("q", 2)), (18, ("k", 4)),
            (32, ("q", 3)), (34, ("k", 5)), (36, ("k", 6)),
            (48, ("k", 7)), (50, ("q", 4)), (52, ("v", 4)),
            (62, ("v", 5)), (64, ("q", 5)), (66, ("v", 6)), (68, ("v", 7)),
            (80, ("q", 6)),
            (96, ("q", 7)),
        ]
        for g, u in USCHED:
            at(g, "unit", u)
        for g in range(max(events) + 1):
            ev = events.get(g, {})
            for vg in ev.get("pv", []):
                next(vg, None)
            for og in ev.get("po", []):
                next(og, None)
            for u in ev.get("unit", []):
                unit(*u, veng=nc.vector if g < 80 else None)
            for pg in ev.get("produce", []):
                next(pg, None)
    _fixup_wait_limits(nc)
    return nc


def _postbl(tsl):
    """Map a token slice to the position slice in the [128, N] pos tables."""
    start, stop = tsl.start, tsl.stop
    return slice(start % N, (start % N) + (stop - start))


def _prep_inputs(hidden_states, cos, sin, w_qkv, norm_q_w, norm_k_w, w_out):
    bf = ml_dtypes.bfloat16
    hid = np.ascontiguousarray(
        np.asarray(hidden_states, np.float32).reshape(T, DIM).T).astype(bf)
    # pack: [tc] -> [128, 8*512] with fc-major columns
    hid_tiled = np.ascontiguousarray(
        hid.reshape(8, 128, 8, 512).transpose(2, 1, 0, 3)).reshape(
            8, 128, 8 * 512)
    cosf = np.asarray(cos, np.float32)     # [N, 64]
    sinf = np.asarray(sin, np.float32)
    wqkv = np.asarray(w_qkv, np.float32)
    woutf = np.asarray(w_out, np.float32)
    wqn = np.asarray(norm_q_w, np.float32)
    wkn = np.asarray(norm_k_w, np.float32)

    # rot matrix lhsT: lhsT[j, d] = sigma(d) if j == pi(d) else 0
    rot = np.zeros((128, 128), np.float32)
    for d in range(128):
        dl = d % 64
        base = d - dl
        pi = base + (dl + 32) % 64
        sg = -1.0 if dl < 32 else 1.0
        rot[pi, d] = sg
    identm = np.eye(128, dtype=np.float32)

    # position tables, feature-major, stacked for 2 heads
    cosT = cosf.T                       # [64, N]
    sinpT = np.empty_like(sinf.T)       # sinp[j, n] = sin[n, (j+32)%64]
    for j in range(64):
        sinpT[j] = sinf[:, (j + 32) % 64]
    cosw = np.vstack([cosT, cosT]).astype(bf)
    sinpw = np.vstack([sinpT, sinpT]).astype(bf)

    # ssq lhsT: [k, m] = (head(k)==head(m)) / w(m)^2
    def ssq_mat(w):
        winv = np.where(np.abs(w) > 1e-20, 1.0 / (w * w), 0.0)
        m = np.zeros((128, 128), np.float32)
        for mm in range(128):
            h = mm // 64
            m[h * 64:(h + 1) * 64, mm] = winv[mm % 64]
        return m

    ssq_q = ssq_mat(wqn).astype(bf)
    ssq_k = ssq_mat(wkn).astype(bf)

    in_maps = []
    for c in range(NCORES):
        hA, hB = 2 * c, 2 * c + 1
        cols = np.r_[hA * HD:(hA + 1) * HD, hB * HD:(hB + 1) * HD]
        def tile_w(wm):
            return np.ascontiguousarray(
                wm.reshape(8, 128, 128).transpose(1, 0, 2).reshape(
                    128, 1024)).astype(bf)

        m = {
            "ht": hid_tiled,
            "wq": tile_w(wqkv[:, cols]),
            "wk": tile_w(wqkv[:, DIM + cols]),
            "wv": tile_w(wqkv[:, 2 * DIM + cols]),
            "wo": np.ascontiguousarray(
                woutf[c * 128:(c + 1) * 128, :]).astype(bf),
            "cosw": cosw,
            "sinpw": sinpw,
            "rotm": rot.astype(bf),
            "ident": identm.astype(bf),
            "ssqq": ssq_q,
            "ssqk": ssq_k,
        }
        in_maps.append(m)
    return in_maps


def kernel(hidden_states, cos, sin, w_qkv, norm_q_w, norm_k_w, w_out):
    if "nc" not in _CACHE:
        _CACHE["nc"] = build_graph()
    nc = _CACHE["nc"]
    in_maps = _prep_inputs(hidden_states, cos, sin, w_qkv, norm_q_w,
                           norm_k_w, w_out)
    trace = bool(int(os.environ.get("KERNEL_TRACE", "0")))
    res = run_bass_kernel_spmd(nc, in_maps, core_ids=list(range(NCORES)),
                               trace=trace)
    _CACHE["last_result"] = res
    outs = res.results
    total = np.zeros((T, DIM), np.float32)
    for m in outs:
        total += np.asarray(m["out"], dtype=np.float32)
    return total.reshape(B, N, DIM)
